# revision 85
# baseline (speedup 1.0000x reference)
"""NatPN radial-flow posterior kernel for Trainium2, 8 NeuronCores (SPMD).

Strategy (class-sharded "expert parallel", lazy-correction flow):
  * Radial flow z' = z + beta*h*(z - z0), h = 1/(alpha + r), r = |z - z0| is
    run in coefficient space z_t = A_t * w_t, w_t = x - sum_k u_k z0_k.
    Per step we need only the scalar dot d1 = w . z0_t, recovered LAZILY:
        d1_t = m0[t] - sum_{k<t} u_k G[k, t]
    with m0 = x @ z0^T (tensor engine) and G = z0 @ z0^T (host). The inner
    product runs as one bf16 2x-mode multiply (u-history and G rows both have
    packed last dims) plus a bf16 add-tree + small tensor_reduce — about half
    the vector-engine cost of eagerly maintaining the m table.
  * State per (class, sample): qw = |w|^2, A = prod(1+bh) (which IS the P
    determinant product), Q = prod(1+alpha*beta*h^2). The qw/Q updates run on
    the otherwise-idle GPSIMD engine; r = sqrt(...) on the scalar engine.
  * 8 cores x 13 classes (100 real + 4 padded with freq -> -1e30), classes
    split into two pipelined groups (7+6) so engines overlap.
  * Epilogue: local max / sum-exp / label-masked partials, one AllToAll
    (96 KB), then each core finishes logsumexp + softmax + Dirichlet update
    for its own 1024-sample slice and writes rows [1024k, 1024(k+1)).
"""
import os
import numpy as np

import concourse.bass as bass
import concourse.bacc as bacc
import concourse.mybir as mybir
from concourse import tile
from concourse.bass_utils import run_bass_kernel_spmd
from concourse.hw_specs import get_activation_tables
from concourse.tile_rust import add_dep_helper

F = mybir.dt.float32
BF = mybir.dt.bfloat16
AF = mybir.ActivationFunctionType
OP = mybir.AluOpType
AX = mybir.AxisListType

NCORES = 8
N, D, C, T = 8192, 64, 100, 30
CP = 13            # classes per core (padded)
S = 64             # sample groups of 128 (N = 128 * S)
SL = 8             # sample groups per core in the epilogue slice
LOG_EV_CLAMP = 10.0
EV_BUDGET = 0.5 * D * float(np.log(4.0 * np.pi))
NEG_HALF_DLOG2PI = -0.5 * D * float(np.log(2.0 * np.pi))
TES = 8            # s_t on DVE below this step
TEQ = 4            # qw/dd chain on DVE below this step
PAD_NEGINF = -1.0e30

_CACHE = {}


def _class_split():
    """cores 0-3 get 13 real classes, cores 4-7 get 12 real + 1 pad."""
    out = []
    off = 0
    for k in range(NCORES):
        cnt = 13 if k < 4 else 12
        cls = list(range(off, off + cnt))
        off += cnt
        real = [True] * cnt
        while len(cls) < CP:
            cls.append(0)
            real.append(False)
        out.append((cls, real))
    assert off == C
    return out


def build_program():
    nc = bacc.Bacc("TRN2", target_bir_lowering=False, debug=False,
                   num_devices=NCORES)

    xbf = nc.dram_tensor("xbf", [D, N], BF, kind="ExternalInput")
    xslice = nc.dram_tensor("xslice", [D + 1, 128 * SL], F, kind="ExternalInput")
    Wb = nc.dram_tensor("Wb", [D + 1, C], F, kind="ExternalInput")
    xsq = nc.dram_tensor("xsq", [128, S], F, kind="ExternalInput")
    z0T = nc.dram_tensor("z0T", [D, CP * T], BF, kind="ExternalInput")
    Gb = nc.dram_tensor("Gb", [CP, 128, T * T], BF, kind="ExternalInput")
    # alpha | beta | n0sq | ab stacked into one tensor -> one DMA
    tabs_r = nc.dram_tensor("tabs_r", [128, 4 * CP * T], F, kind="ExternalInput")
    cadd_r = nc.dram_tensor("cadd_r", [128, CP], F, kind="ExternalInput")
    corr_in = nc.dram_tensor("corr", [128, SL], F, kind="ExternalInput")
    masksb = nc.dram_tensor("masksb", [128, CP * S], F, kind="ExternalInput")
    out_d = nc.dram_tensor("out", [128 * SL, C + 1], F, kind="ExternalOutput")

    GROUPS = [(0, 7), (7, 6)]   # (class offset, count)

    with tile.TileContext(nc) as tc:
        with tc.tile_pool(name="const", bufs=1) as cp_, \
             tc.tile_pool(name="st", bufs=1) as stp, \
             tc.tile_pool(name="pm", bufs=6, space="PSUM") as pmp, \
             tc.tile_pool(name="pl", bufs=2, space="PSUM") as plp, \
             tc.tile_pool(name="dram", bufs=1, space="DRAM") as dp:

            # ---- resident constants / state ----
            # (DMA issue order matters: x chunk + z0 first so the tensor
            #  engine starts immediately; big Gb load afterwards)
            tabs_sb = cp_.tile([128, 4 * CP * T], F)
            CT = CP * T
            alpha_sb = tabs_sb[:, 0:CT]
            beta_sb = tabs_sb[:, CT:2 * CT]
            n0sq_sb = tabs_sb[:, 2 * CT:3 * CT]
            ab_sb = tabs_sb[:, 3 * CT:4 * CT]
            xsq_sb = cp_.tile([128, S], F)
            cadd_sb = cp_.tile([128, CP], F)
            gb_sb = cp_.tile([128, CP * T * T], BF)
            gb4 = gb_sb.rearrange("p (c t j) -> p c t j", c=CP, j=T)

            # per-(c,s) f32 state, CP-wide (groups use class sub-ranges)
            qw_t = stp.tile([128, CP, S], F)
            A_t = stp.tile([128, CP, S], F)
            Q_t = stp.tile([128, CP, S], F)
            d1_t = stp.tile([128, CP, S], F)
            rs_t = stp.tile([128, CP, S], F)
            bh_t = stp.tile([128, CP, S], F)
            rs2_t = stp.tile([128, CP, S], F)

            last_flow_act = None

            with tc.tile_pool(name="big", bufs=1) as bigp, \
                 tc.tile_pool(name="sc", bufs=1) as sc:
                m0 = bigp.tile([128, T * CP * S], BF)        # t-major: [t, c, s]
                m04 = m0.rearrange("p (t c s) -> p t c s", t=T, s=S)
                uacc = bigp.tile([128, CP * S * T], BF)      # k-minor: [c, s, k]
                uacc4 = uacc.rearrange("p (c s k) -> p c s k", s=S, k=T)
                wtl = bigp.tile([128, 7 * S * (T - 1)], BF)  # shared scratch

                # ---- init: m0 = x @ z0^T on the tensor engine ----
                # x loaded in 2048-sample chunks; group A's classes complete
                # first so its flow steps start while group B still inits.
                with tc.tile_pool(name="init", bufs=1) as ip:
                    xch0 = ip.tile([D, 128 * 16], BF, name="xch0")
                    nc.sync.dma_start(xch0[:], xbf[:, 0:2048])
                    z0all = ip.tile([D, CP * T], BF)
                    nc.sync.dma_start(z0all[:], z0T[:])
                    # remaining constants (needed only once flow starts)
                    nc.sync.dma_start(tabs_sb[:], tabs_r[:])
                    nc.sync.dma_start(xsq_sb[:], xsq[:])
                    nc.sync.dma_start(cadd_sb[:], cadd_r[:])
                    nc.sync.dma_start(
                        gb_sb[:].rearrange("p (c g) -> p c g", c=CP),
                        Gb[:].rearrange("c p g -> p c g"))
                    cp_rr = [0]
                    for (c0g, Gg) in GROUPS:
                        # class blocks of up to 4 share one rhs (wider
                        # matmuls -> 3.5x fewer PE instructions)
                        blocks = []
                        cc = c0g
                        while cc < c0g + Gg:
                            nb = min(4, c0g + Gg - cc)
                            blocks.append((cc, nb))
                            cc += nb
                        for i in range(S // 16):
                            if (c0g, i) == (0, 0):
                                xch = xch0
                            else:
                                xch = ip.tile([D, 128 * 16], BF,
                                              name=f"xch{i % 2}")
                                nc.sync.dma_start(
                                    xch[:], xbf[:, 2048 * i:2048 * (i + 1)])
                            for (cb, nb) in blocks:
                                w_ = T * nb          # rhs cols
                                jm = 480 // w_       # samples per PSUM tile
                                for j0 in range(0, 16, jm):
                                    jn = min(jm, 16 - j0)
                                    pm = pmp.tile([128, jm * w_], F, name="pm")
                                    for j in range(jn):
                                        s_ = 128 * (j0 + j)
                                        nc.tensor.matmul(
                                            pm[:, w_ * j:w_ * (j + 1)],
                                            lhsT=xch[:, s_:s_ + 128],
                                            rhs=z0all[:, T * cb:T * cb + w_],
                                            start=True, stop=True)
                                    # pm dims (j, c, t) -> m0[t, c, 16i+j0+j]
                                    # copies round-robin across DVE/ACT/Pool
                                    # (all three idle during init)
                                    s0 = 16 * i + j0
                                    cp_fn = (nc.vector.tensor_copy,
                                             nc.scalar.copy)[cp_rr[0] % 2]
                                    cp_rr[0] += 1
                                    cp_fn(
                                        m04[:, :, cb:cb + nb,
                                            s0:s0 + jn].rearrange(
                                                "p t c s -> p s c t"),
                                        pm[:, :jn * w_].rearrange(
                                            "p (s c t) -> p s c t",
                                            c=nb, t=T))

                # state init
                nc.vector.tensor_copy(
                    qw_t[:, :, :],
                    xsq_sb[:, None, :].broadcast_to((128, CP, S)))
                nc.gpsimd.memset(A_t[:], 1.0)
                nc.gpsimd.memset(Q_t[:], 1.0)

                # =======================  flow phase  =======================
                # Lookahead correction: during step t we precompute
                #   corrpre(t+1) = sum_{k<t} u_k * G[k, t+1]
                # (bulk multiply + bf16 add-tree, off the serial chain) and
                #   s1(t+1) = m0[t+1] - corrpre(t+1).
                # The inter-step chain then only carries the last column:
                #   d1(t+1) = s1(t+1) - u_t * G[t, t+1].
                # The r^2 / qw / Q scalar chains run on the GPSIMD engine.
                gstate = [dict() for _ in GROUPS]

                def gslice(g):
                    c0, G_ = GROUPS[g]
                    return (slice(None), slice(c0, c0 + G_), slice(None))

                def flow_head(g, t):
                    c0, G_ = GROUPS[g]
                    st = gstate[g]
                    FGS = G_ * S
                    d1 = d1_t[gslice(g)]
                    m0col = m04[:, t, c0:c0 + G_, :]
                    u4 = uacc4[:, c0:c0 + G_, :, :]

                    def tl(name):
                        return sc.tile([128, G_, S], F, name=f"{name}{g}")

                    # ---- corr = sum_{k<t} u_k * G[k, t]  (lazy dot) ----
                    if t == 0:
                        nc.vector.tensor_copy(d1, m0col)
                    else:
                        w4 = wtl[:, :FGS * t].rearrange(
                            "p (c s k) -> p c s k", c=G_, k=t)
                        gview = gb4[:, c0:c0 + G_, t, 0:t][:, :, None, :]
                        nc.vector.tensor_tensor(
                            out=w4, in0=u4[:, :, :, 0:t],
                            in1=gview.broadcast_to((128, G_, S, t)), op=OP.mult)
                        k = t
                        if k > 2:
                            p2 = 1 << (k.bit_length() - 1)
                            if p2 == k:
                                p2 //= 2
                            nc.vector.tensor_tensor(
                                out=w4[:, :, :, 0:k - p2],
                                in0=w4[:, :, :, 0:k - p2],
                                in1=w4[:, :, :, p2:k], op=OP.add)
                            k = p2
                            while k > 2:
                                h = k // 2
                                nc.vector.tensor_tensor(
                                    out=w4[:, :, :, 0:h], in0=w4[:, :, :, 0:h],
                                    in1=w4[:, :, :, h:k], op=OP.add)
                                k = h
                        nc.vector.tensor_tensor(
                            out=d1, in0=m0col, in1=w4[:, :, :, 0],
                            op=OP.subtract)
                        if k == 2:
                            nc.vector.tensor_tensor(
                                out=d1, in0=d1, in1=w4[:, :, :, 1],
                                op=OP.subtract)
                    dd = tl("dd")
                    de = nc.vector if t < TEQ else nc.gpsimd
                    de.tensor_tensor(out=dd[:], in0=d1, in1=d1, op=OP.add)
                    st["dd"] = dd

                def flow_mid(g, t):
                    nonlocal last_flow_act
                    c0, G_ = GROUPS[g]
                    st = gstate[g]
                    dd = st["dd"]
                    sl3 = gslice(g)
                    qw = qw_t[sl3]
                    A = A_t[sl3]
                    Q = Q_t[sl3]
                    d1 = d1_t[sl3]
                    rs = rs_t[sl3]
                    bh = bh_t[sl3]
                    rs2 = rs2_t[sl3]
                    u4 = uacc4[:, c0:c0 + G_, :, :]

                    def tl(name):
                        return sc.tile([128, G_, S], F, name=f"{name}{g}")

                    # ---- r2 = A*(A*qw - 2*d1) + n0sq ----
                    g1 = tl("g1")
                    nc.vector.tensor_tensor(out=g1[:], in0=A, in1=qw,
                                            op=OP.mult)
                    g2 = tl("g2")
                    nc.vector.scalar_tensor_tensor(g2[:], d1, -2.0, g1[:],
                                                   op0=OP.mult, op1=OP.add)
                    r2m = tl("g1")      # g1 dead after g2
                    nc.vector.tensor_tensor(out=r2m[:], in0=A, in1=g2[:],
                                            op=OP.mult)
                    r = tl("lc")        # lc dead after d1
                    for ci in range(G_):
                        ct = T * (c0 + ci) + t
                        last_flow_act = nc.scalar.activation(
                            r[:, ci, :], r2m[:, ci, :], AF.Sqrt,
                            bias=n0sq_sb[:, ct:ct + 1], scale=1.0)
                    # s = r + alpha ; rs = 1/s ; bh = beta*rs
                    # early steps are chain-bound: keep s_t on DVE there
                    s_t = tl("s_t")
                    if t < TES:
                        av = alpha_sb.rearrange("p (c t) -> p c t", t=T)[
                            :, c0:c0 + G_, t][:, :, None]
                        nc.vector.tensor_tensor(
                            out=s_t[:], in0=r[:],
                            in1=av.broadcast_to((128, G_, S)), op=OP.add)
                    else:
                        for ci in range(G_):
                            ct = T * (c0 + ci) + t
                            nc.scalar.activation(
                                s_t[:, ci, :], r[:, ci, :], AF.Identity,
                                bias=alpha_sb[:, ct:ct + 1], scale=1.0)
                    nc.vector.reciprocal_approx_fast(rs, s_t[:])
                    bv = beta_sb.rearrange("p (c t) -> p c t", t=T)[
                        :, c0:c0 + G_, t][:, :, None]
                    nc.vector.tensor_tensor(
                        out=bh, in0=rs,
                        in1=bv.broadcast_to((128, G_, S)), op=OP.mult)
                    # Q *= 1 + ab*rs^2   (scalar engine square, gpsimd chain)
                    nc.scalar.activation(rs2, rs, AF.Square)
                    k1 = tl("s_t")      # s_t dead after rs
                    abv = ab_sb.rearrange("p (c t) -> p c t", t=T)[
                        :, c0:c0 + G_, t][:, :, None]
                    nc.gpsimd.tensor_tensor(
                        out=k1[:], in0=rs2,
                        in1=abv.broadcast_to((128, G_, S)), op=OP.mult)
                    # Q *= (1 + v)  as  Q += Q*v  (no scalar ops on Pool)
                    k2 = tl("g1")
                    nc.gpsimd.tensor_tensor(out=k2[:], in0=Q, in1=k1[:],
                                            op=OP.mult)
                    nc.gpsimd.tensor_tensor(out=Q, in0=Q, in1=k2[:], op=OP.add)
                    # A' = (1+bh)*A  (in place)
                    nc.vector.scalar_tensor_tensor(A, bh, 1.0, A,
                                                   op0=OP.add, op1=OP.mult)
                    # ut = bh / A'  (stored bf16 into the u history)
                    rA = tl("g2")       # g2 dead after r2m
                    nc.vector.reciprocal_approx_fast(rA[:], A)
                    ut = u4[:, :, :, t]
                    nc.vector.tensor_tensor(out=ut, in0=bh, in1=rA[:],
                                            op=OP.mult)
                    # qw' = qw + ut*(ut*Gtt - 2*d1)
                    # (gpsimd once steps are long enough to hide it)
                    qe = nc.gpsimd
                    gttv = gb4[:, c0:c0 + G_, t, t][:, :, None]
                    h1 = tl("h1")
                    qe.tensor_tensor(
                        out=h1[:], in0=ut,
                        in1=gttv.broadcast_to((128, G_, S)), op=OP.mult)
                    h2 = tl("h2")
                    qe.tensor_tensor(out=h2[:], in0=h1[:], in1=dd[:],
                                     op=OP.subtract)
                    h3 = tl("h1")       # h1 dead after h2
                    qe.tensor_tensor(out=h3[:], in0=ut, in1=h2[:],
                                     op=OP.mult)
                    qe.tensor_tensor(out=qw, in0=qw, in1=h3[:],
                                     op=OP.add)

                for t in range(T):
                    for g in range(len(GROUPS)):
                        flow_head(g, t)
                    for g in range(len(GROUPS)):
                        flow_mid(g, t)

            # =========================  epilogue  =========================
            # Pin all epilogue ACT work behind a single natural_log_exp table
            # load (Sqrt/Ln/Exp live in different sets).
            nle_id = list(get_activation_tables(nc.m.arch)).index(
                "natural_log_exp_and_others")
            tbl_load = mybir.InstLoadActFuncSet(
                name=f"I-{nc.next_id()}", act_func_set_id=nle_id, ins=[], outs=[])
            tl_bi = nc.scalar.add_instruction(tbl_load)
            add_dep_helper(tl_bi.ins, last_flow_act.ins, True,
                           "table load after flow phase")

            def act_pinned(out, in_, func, **kw):
                bi = nc.scalar.activation(out, in_, func, **kw)
                add_dep_helper(bi.ins, tl_bi.ins, True, "epilogue act after load")
                return bi

            with tc.tile_pool(name="epi", bufs=1) as ep:
                lpw = ep.tile([128, CP * S], F)
                lpw3 = lpw.rearrange("p (c s) -> p c s", s=S)
                # lpw = -0.5*A^2*qw + 63*ln(A) + ln(Q) + cadd
                # za/zq/zqc only need flow state -> gpsimd, ahead of the Lns
                za = ep.tile([128, CP, S], F)
                nc.vector.tensor_tensor(out=za[:, :, :], in0=A_t[:, :, :],
                                        in1=A_t[:, :, :], op=OP.mult)
                zq = ep.tile([128, CP, S], F)
                nc.vector.tensor_tensor(out=zq[:, :, :], in0=za[:, :, :],
                                        in1=qw_t[:, :, :], op=OP.mult)
                cv = cadd_sb[:, :, None]
                zqc = ep.tile([128, CP, S], F)
                nc.vector.scalar_tensor_tensor(
                    zqc[:, :, :], zq[:, :, :], -0.5,
                    cv.broadcast_to((128, CP, S)),
                    op0=OP.mult, op1=OP.add)
                l1 = ep.tile([128, CP * S], F)
                act_pinned(l1[:], A_t[:, :, :].rearrange("p c s -> p (c s)"),
                           AF.Ln)
                l2 = ep.tile([128, CP * S], F)
                act_pinned(l2[:], Q_t[:, :, :].rearrange("p c s -> p (c s)"),
                           AF.Ln)
                w1 = ep.tile([128, CP * S], F)
                nc.vector.scalar_tensor_tensor(w1[:], l1[:], float(D - 1), l2[:],
                                               op0=OP.mult, op1=OP.add)
                nc.vector.tensor_tensor(
                    out=lpw3, in0=w1.rearrange("p (c s) -> p c s", s=S),
                    in1=zqc[:, :, :], op=OP.add)

                lpw_perm = lpw.rearrange("p (c s) -> p s c", s=S)
                mscl = ep.tile([128, 3, S], F)
                mx = mscl[:, 0, :]
                se = mscl[:, 1, :]
                clsl = mscl[:, 2, :]
                nc.vector.tensor_reduce(mx, lpw_perm, axis=AX.X, op=OP.max)
                exs = ep.tile([128, CP * S], F)
                exs3 = exs.rearrange("p (c s) -> p c s", s=S)
                mx_b = mx[:, None, :].broadcast_to((128, CP, S))
                nc.vector.tensor_tensor(out=exs3, in0=lpw3[:, :, :], in1=mx_b,
                                        op=OP.subtract)
                act_pinned(exs[:], exs[:], AF.Exp)
                nc.vector.tensor_reduce(
                    se, exs.rearrange("p (c s) -> p s c", s=S),
                    axis=AX.X, op=OP.add)
                msk_sb = ep.tile([128, CP * S], F)
                nc.sync.dma_start(msk_sb[:], masksb[:])
                gsum = ep.tile([128, CP * S], F)   # own buffer: the mask
                # path runs in parallel with the sum-exp path
                nc.vector.tensor_tensor(out=gsum[:], in0=msk_sb[:], in1=lpw[:],
                                        op=OP.mult)
                nc.vector.tensor_reduce(
                    clsl, gsum.rearrange("p (c s) -> p s c", s=S),
                    axis=AX.X, op=OP.add)

                # ---- AllToAll: ccin[j] = (mx, se, cls) for sample-slice j ----
                ccin = dp.tile([NCORES, 3, 128 * SL], F)
                ccout = dp.tile([NCORES, 3, 128 * SL], F)
                ccin_v = ccin.rearrange("r t (p s) -> t p r s", p=128)
                for ti in range(3):
                    nc.sync.dma_start(
                        ccin_v[ti],
                        mscl[:, ti, :].rearrange("p (r s) -> p r s", s=SL))
                nc.gpsimd.collective_compute(
                    "AllToAll", OP.bypass,
                    replica_groups=[list(range(NCORES))],
                    ins=[ccin.opt()], outs=[ccout.opt()],
                )
                # ---- logits path fills the AllToAll wait ----
                xsl_sb = ep.tile([D + 1, 128 * SL], F)
                nc.sync.dma_start(xsl_sb[:], xslice[:])
                Wb_sb = ep.tile([D + 1, C], F)
                nc.sync.dma_start(Wb_sb[:], Wb[:])
                lg = ep.tile([128, SL * C], F)
                for j in range(SL):
                    pl = plp.tile([128, C], F)
                    nc.tensor.matmul(pl[:],
                                     lhsT=xsl_sb[:, 128 * j:128 * (j + 1)],
                                     rhs=Wb_sb[:], start=True, stop=True)
                    nc.scalar.copy(lg[:, C * j:C * (j + 1)], pl[:])
                lg3 = lg.rearrange("p (s c) -> p s c", c=C)
                ml = ep.tile([128, SL], F)
                nc.vector.tensor_reduce(ml[:], lg3, axis=AX.X, op=OP.max)
                ml_b = ml[:, :, None].broadcast_to((128, SL, C))
                nc.vector.tensor_tensor(out=lg3, in0=lg3, in1=ml_b,
                                        op=OP.subtract)
                act_pinned(lg[:], lg[:], AF.Exp)
                ssum = ep.tile([128, SL], F)
                nc.vector.tensor_reduce(ssum[:], lg3, axis=AX.X, op=OP.add)
                rsum = ep.tile([128, SL], F)
                rscr = ep.tile([128, SL], F)
                nc.vector.reciprocal_approx_accurate(rsum[:], ssum[:], rscr[:])

                ccout_v = ccout.rearrange("r t (p s) -> t p r s", p=128)
                cco = ep.tile([128, 3, NCORES, SL], F)
                for ti in range(3):
                    nc.sync.dma_start(cco[:, ti], ccout_v[ti])
                mxg = cco[:, 0]
                seg = cco[:, 1]
                clg = cco[:, 2]

                # ---- global combine for our slice ----
                M = ep.tile([128, SL], F)
                nc.vector.tensor_reduce(M[:], mxg.rearrange("p r s -> p s r"),
                                        axis=AX.X, op=OP.max)
                esh = ep.tile([128, NCORES * SL], F)
                esh3 = esh.rearrange("p (r s) -> p r s", s=SL)
                M_b = M[:, None, :].broadcast_to((128, NCORES, SL))
                nc.vector.tensor_tensor(out=esh3, in0=mxg, in1=M_b,
                                        op=OP.subtract)
                act_pinned(esh[:], esh[:], AF.Exp)
                wsum = ep.tile([128, NCORES * SL], F)
                nc.vector.tensor_tensor(out=wsum[:], in0=esh[:], in1=seg.rearrange("p r s -> p (r s)"),
                                        op=OP.mult)
                Sg = ep.tile([128, SL], F)
                nc.vector.tensor_reduce(
                    Sg[:], wsum.rearrange("p (r s) -> p s r", s=SL),
                    axis=AX.X, op=OP.add)
                lse = ep.tile([128, SL], F)
                act_pinned(lse[:], Sg[:], AF.Ln)
                nc.vector.tensor_tensor(out=lse[:], in0=lse[:], in1=M[:],
                                        op=OP.add)
                clsf = ep.tile([128, SL], F)
                nc.vector.tensor_reduce(clsf[:], clg.rearrange("p r s -> p s r"),
                                        axis=AX.X, op=OP.add)
                corr_sb = ep.tile([128, SL], F)
                nc.sync.dma_start(corr_sb[:], corr_in[:])
                nc.vector.tensor_tensor(out=clsf[:], in0=clsf[:], in1=corr_sb[:],
                                        op=OP.subtract)
                lev = ep.tile([128, SL], F)
                nc.vector.tensor_scalar(out=lev[:], in0=lse[:],
                                        scalar1=EV_BUDGET,
                                        scalar2=LOG_EV_CLAMP, op0=OP.add,
                                        op1=OP.min)
                ev = ep.tile([128, SL], F)
                act_pinned(ev[:], lev[:], AF.Exp)

                # ---- combine evidence with precomputed softmax ----
                evn = ep.tile([128, SL], F)
                nc.vector.tensor_tensor(out=evn[:], in0=ev[:], in1=rsum[:],
                                        op=OP.mult)
                evn_b = evn[:, :, None].broadcast_to((128, SL, C))
                t1 = lg  # in-place: exp(logits) no longer needed afterwards
                t13 = lg3
                nc.vector.tensor_tensor(out=t13, in0=lg3, in1=evn_b, op=OP.mult)
                la = gsum[:, :SL * C]  # gsum dead after the cls reduce
                act_pinned(la[:], t1[:], AF.Ln, bias=1.0)
                # accurate log1p for small x: x*(1 + x*(-1/2 + x/3)) when x<0.01
                h1e = ep.tile([128, SL * C], F)
                nc.vector.tensor_scalar(out=h1e[:], in0=t1[:], scalar1=1.0 / 3.0,
                                        scalar2=-0.5, op0=OP.mult, op1=OP.add)
                nc.vector.tensor_tensor(out=h1e[:], in0=h1e[:], in1=t1[:],
                                        op=OP.mult)
                nc.vector.tensor_scalar_add(h1e[:], h1e[:], 1.0)
                nc.vector.tensor_tensor(out=h1e[:], in0=h1e[:], in1=t1[:],
                                        op=OP.mult)
                h2e = h1e
                lmask = ep.tile([128, SL * C], mybir.dt.uint8)
                nc.vector.tensor_scalar(out=lmask[:], in0=t1[:], scalar1=0.01,
                                        scalar2=None, op0=OP.is_lt)
                nc.vector.select(la[:], lmask[:], h2e[:], la[:])

                ob = lpw[:, :SL * (C + 1)]  # lpw dead after gsum
                ob3 = ob.rearrange("p (s c) -> p s c", c=C + 1)
                la3 = la.rearrange("p (s c) -> p s c", c=C)
                out_v = out_d.rearrange("(s p) c -> p s c", p=128)
                H = SL // 2
                nc.vector.tensor_copy(ob3[:, :H, 0:C], la3[:, :H, :])
                nc.vector.tensor_copy(ob3[:, :H, C:C + 1], clsf[:, :H, None])
                nc.sync.dma_start(out_v[:, :H], ob3[:, :H, :])
                nc.vector.tensor_copy(ob3[:, H:, 0:C], la3[:, H:, :])
                nc.vector.tensor_copy(ob3[:, H:, C:C + 1], clsf[:, H:, None])
                nc.sync.dma_start(out_v[:, H:], ob3[:, H:, :])

    nc.finalize()
    return nc


def _softplus(v):
    return np.log1p(np.exp(-np.abs(v))) + np.maximum(v, 0)


def host_prep(x, labels, labels_frequency, z0, alpha_prime, beta_prime, W, b):
    import ml_dtypes
    x = np.asarray(x, np.float32)
    labels = np.asarray(labels).astype(np.int64)
    freq = np.asarray(labels_frequency, np.float32)
    z0 = np.asarray(z0, np.float32)
    alpha = _softplus(np.asarray(alpha_prime, np.float32)).astype(np.float32)
    beta = (-alpha + _softplus(np.asarray(beta_prime, np.float32))).astype(np.float32)
    W = np.asarray(W, np.float32)
    b = np.asarray(b, np.float32)

    xaugT = np.concatenate([x.T, np.ones((1, N), np.float32)], axis=0)  # [65, N]
    xbf = np.ascontiguousarray(x.T).astype(ml_dtypes.bfloat16)          # [D, N]
    Wb = np.concatenate([W, b[None, :]], axis=0).astype(np.float32)    # [65, C]
    xsq = np.sum(x * x, axis=1).astype(np.float32).reshape(S, 128).T   # [128, S]
    logfreq = np.log(freq).astype(np.float32)
    lab_ps = labels.reshape(S, 128).T                                  # [128, S]

    ones128 = np.ones((128, 1), np.float32)
    in_maps = []
    for k, (cls, real) in enumerate(_class_split()):
        z0c = z0[cls]                                   # [CP, T, D]
        alc = alpha[cls]                                # [CP, T]
        bec = beta[cls]
        G = np.einsum('cij,ckj->cik', z0c, z0c).astype(np.float32)   # [CP,T,T]
        n0 = np.sum(z0c * z0c, axis=2).astype(np.float32)            # [CP, T]
        Gb = np.broadcast_to(
            G.astype(ml_dtypes.bfloat16).reshape(CP, 1, T * T),
            (CP, 128, T * T)).copy()
        ab = (alc * bec).astype(np.float32)
        tabs = np.concatenate([alc.reshape(-1), bec.reshape(-1),
                               n0.reshape(-1), ab.reshape(-1)])
        tabs_rk = np.broadcast_to(tabs.reshape(1, 4 * CP * T),
                                  (128, 4 * CP * T)).copy()
        cadd = np.array([(logfreq[c] + NEG_HALF_DLOG2PI) if r else PAD_NEGINF
                         for c, r in zip(cls, real)], np.float32)
        cadd_rk = (ones128 * cadd[None, :]).astype(np.float32)
        msk = np.zeros((128, CP, S), np.float32)
        for i, (c, r) in enumerate(zip(cls, real)):
            if r:
                msk[:, i, :] = (lab_ps == c)
        sl = slice(1024 * k, 1024 * (k + 1))
        corr_k = logfreq[labels[sl]].reshape(SL, 128).T.astype(np.float32)
        in_maps.append(dict(
            xbf=xbf, xslice=np.ascontiguousarray(xaugT[:, sl]), Wb=Wb,
            xsq=xsq,
            z0T=np.ascontiguousarray(z0c.transpose(2, 0, 1)).reshape(
                D, CP * T).astype(ml_dtypes.bfloat16),
            Gb=Gb, tabs_r=tabs_rk,
            cadd_r=cadd_rk, corr=corr_k,
            masksb=msk.reshape(128, CP * S),
        ))
    return in_maps


def kernel(**inputs) -> np.ndarray:
    if "nc" not in _CACHE:
        _CACHE["nc"] = build_program()
    nc = _CACHE["nc"]
    in_maps = host_prep(**inputs)
    if os.environ.get("KERNEL_SIM"):
        from concourse.bass_interp import MultiCoreSim
        sim = MultiCoreSim(nc, NCORES)
        for k in range(NCORES):
            for name, arr in in_maps[k].items():
                sim.cores[k].tensor(name)[:] = arr
        sim.simulate()
        outs = [np.array(sim.cores[k].tensor("out")) for k in range(NCORES)]
    else:
        res = run_bass_kernel_spmd(nc, in_maps, list(range(NCORES)))
        outs = [res.results[k]["out"] for k in range(NCORES)]
    return np.concatenate(outs, axis=0)


# revision 86
# speedup vs baseline: 1.0025x; 1.0025x over previous
"""NatPN radial-flow posterior kernel for Trainium2, 8 NeuronCores (SPMD).

Strategy (class-sharded "expert parallel", lazy-correction flow):
  * Radial flow z' = z + beta*h*(z - z0), h = 1/(alpha + r), r = |z - z0| is
    run in coefficient space z_t = A_t * w_t, w_t = x - sum_k u_k z0_k.
    Per step we need only the scalar dot d1 = w . z0_t, recovered LAZILY:
        d1_t = m0[t] - sum_{k<t} u_k G[k, t]
    with m0 = x @ z0^T (tensor engine) and G = z0 @ z0^T (host). The inner
    product runs as one bf16 2x-mode multiply (u-history and G rows both have
    packed last dims) plus a bf16 add-tree + small tensor_reduce — about half
    the vector-engine cost of eagerly maintaining the m table.
  * State per (class, sample): qw = |w|^2, A = prod(1+bh) (which IS the P
    determinant product), Q = prod(1+alpha*beta*h^2). The qw/Q updates run on
    the otherwise-idle GPSIMD engine; r = sqrt(...) on the scalar engine.
  * 8 cores x 13 classes (100 real + 4 padded with freq -> -1e30), classes
    split into two pipelined groups (7+6) so engines overlap.
  * Epilogue: local max / sum-exp / label-masked partials, one AllToAll
    (96 KB), then each core finishes logsumexp + softmax + Dirichlet update
    for its own 1024-sample slice and writes rows [1024k, 1024(k+1)).
"""
import os
import numpy as np

import concourse.bass as bass
import concourse.bacc as bacc
import concourse.mybir as mybir
from concourse import tile
from concourse.bass_utils import run_bass_kernel_spmd
from concourse.hw_specs import get_activation_tables
from concourse.tile_rust import add_dep_helper

F = mybir.dt.float32
BF = mybir.dt.bfloat16
AF = mybir.ActivationFunctionType
OP = mybir.AluOpType
AX = mybir.AxisListType

NCORES = 8
N, D, C, T = 8192, 64, 100, 30
CP = 13            # classes per core (padded)
S = 64             # sample groups of 128 (N = 128 * S)
SL = 8             # sample groups per core in the epilogue slice
LOG_EV_CLAMP = 10.0
EV_BUDGET = 0.5 * D * float(np.log(4.0 * np.pi))
NEG_HALF_DLOG2PI = -0.5 * D * float(np.log(2.0 * np.pi))
TES = 5            # s_t on DVE below this step
TEQ = 8            # qw/dd chain on DVE below this step
PAD_NEGINF = -1.0e30

_CACHE = {}


def _class_split():
    """cores 0-3 get 13 real classes, cores 4-7 get 12 real + 1 pad."""
    out = []
    off = 0
    for k in range(NCORES):
        cnt = 13 if k < 4 else 12
        cls = list(range(off, off + cnt))
        off += cnt
        real = [True] * cnt
        while len(cls) < CP:
            cls.append(0)
            real.append(False)
        out.append((cls, real))
    assert off == C
    return out


def build_program():
    nc = bacc.Bacc("TRN2", target_bir_lowering=False, debug=False,
                   num_devices=NCORES)

    xbf = nc.dram_tensor("xbf", [D, N], BF, kind="ExternalInput")
    xslice = nc.dram_tensor("xslice", [D + 1, 128 * SL], F, kind="ExternalInput")
    Wb = nc.dram_tensor("Wb", [D + 1, C], F, kind="ExternalInput")
    xsq = nc.dram_tensor("xsq", [128, S], F, kind="ExternalInput")
    z0T = nc.dram_tensor("z0T", [D, CP * T], BF, kind="ExternalInput")
    Gb = nc.dram_tensor("Gb", [CP, 128, T * T], BF, kind="ExternalInput")
    # alpha | beta | n0sq | ab stacked into one tensor -> one DMA
    tabs_r = nc.dram_tensor("tabs_r", [128, 4 * CP * T], F, kind="ExternalInput")
    cadd_r = nc.dram_tensor("cadd_r", [128, CP], F, kind="ExternalInput")
    corr_in = nc.dram_tensor("corr", [128, SL], F, kind="ExternalInput")
    masksb = nc.dram_tensor("masksb", [128, CP * S], F, kind="ExternalInput")
    out_d = nc.dram_tensor("out", [128 * SL, C + 1], F, kind="ExternalOutput")

    GROUPS = [(0, 7), (7, 6)]   # (class offset, count)

    with tile.TileContext(nc) as tc:
        with tc.tile_pool(name="const", bufs=1) as cp_, \
             tc.tile_pool(name="st", bufs=1) as stp, \
             tc.tile_pool(name="pm", bufs=6, space="PSUM") as pmp, \
             tc.tile_pool(name="pl", bufs=2, space="PSUM") as plp, \
             tc.tile_pool(name="dram", bufs=1, space="DRAM") as dp:

            # ---- resident constants / state ----
            # (DMA issue order matters: x chunk + z0 first so the tensor
            #  engine starts immediately; big Gb load afterwards)
            tabs_sb = cp_.tile([128, 4 * CP * T], F)
            CT = CP * T
            alpha_sb = tabs_sb[:, 0:CT]
            beta_sb = tabs_sb[:, CT:2 * CT]
            n0sq_sb = tabs_sb[:, 2 * CT:3 * CT]
            ab_sb = tabs_sb[:, 3 * CT:4 * CT]
            xsq_sb = cp_.tile([128, S], F)
            cadd_sb = cp_.tile([128, CP], F)
            gb_sb = cp_.tile([128, CP * T * T], BF)
            gb4 = gb_sb.rearrange("p (c t j) -> p c t j", c=CP, j=T)

            # per-(c,s) f32 state, CP-wide (groups use class sub-ranges)
            qw_t = stp.tile([128, CP, S], F)
            A_t = stp.tile([128, CP, S], F)
            Q_t = stp.tile([128, CP, S], F)
            d1_t = stp.tile([128, CP, S], F)
            rs_t = stp.tile([128, CP, S], F)
            bh_t = stp.tile([128, CP, S], F)
            rs2_t = stp.tile([128, CP, S], F)

            last_flow_act = None

            with tc.tile_pool(name="big", bufs=1) as bigp, \
                 tc.tile_pool(name="sc", bufs=1) as sc:
                m0 = bigp.tile([128, T * CP * S], BF)        # t-major: [t, c, s]
                m04 = m0.rearrange("p (t c s) -> p t c s", t=T, s=S)
                uacc = bigp.tile([128, CP * S * T], BF)      # k-minor: [c, s, k]
                uacc4 = uacc.rearrange("p (c s k) -> p c s k", s=S, k=T)
                wtl = bigp.tile([128, 7 * S * (T - 1)], BF)  # shared scratch

                # ---- init: m0 = x @ z0^T on the tensor engine ----
                # x loaded in 2048-sample chunks; group A's classes complete
                # first so its flow steps start while group B still inits.
                with tc.tile_pool(name="init", bufs=1) as ip:
                    xch0 = ip.tile([D, 128 * 16], BF, name="xch0")
                    nc.sync.dma_start(xch0[:], xbf[:, 0:2048])
                    z0all = ip.tile([D, CP * T], BF)
                    nc.sync.dma_start(z0all[:], z0T[:])
                    # remaining constants (needed only once flow starts)
                    nc.sync.dma_start(tabs_sb[:], tabs_r[:])
                    nc.sync.dma_start(xsq_sb[:], xsq[:])
                    nc.sync.dma_start(cadd_sb[:], cadd_r[:])
                    nc.sync.dma_start(
                        gb_sb[:].rearrange("p (c g) -> p c g", c=CP),
                        Gb[:].rearrange("c p g -> p c g"))
                    cp_rr = [0]
                    for (c0g, Gg) in GROUPS:
                        # class blocks of up to 4 share one rhs (wider
                        # matmuls -> 3.5x fewer PE instructions)
                        blocks = []
                        cc = c0g
                        while cc < c0g + Gg:
                            nb = min(4, c0g + Gg - cc)
                            blocks.append((cc, nb))
                            cc += nb
                        for i in range(S // 16):
                            if (c0g, i) == (0, 0):
                                xch = xch0
                            else:
                                xch = ip.tile([D, 128 * 16], BF,
                                              name=f"xch{i % 2}")
                                nc.sync.dma_start(
                                    xch[:], xbf[:, 2048 * i:2048 * (i + 1)])
                            for (cb, nb) in blocks:
                                w_ = T * nb          # rhs cols
                                jm = 480 // w_       # samples per PSUM tile
                                for j0 in range(0, 16, jm):
                                    jn = min(jm, 16 - j0)
                                    pm = pmp.tile([128, jm * w_], F, name="pm")
                                    for j in range(jn):
                                        s_ = 128 * (j0 + j)
                                        nc.tensor.matmul(
                                            pm[:, w_ * j:w_ * (j + 1)],
                                            lhsT=xch[:, s_:s_ + 128],
                                            rhs=z0all[:, T * cb:T * cb + w_],
                                            start=True, stop=True)
                                    # pm dims (j, c, t) -> m0[t, c, 16i+j0+j]
                                    # copies round-robin across DVE/ACT/Pool
                                    # (all three idle during init)
                                    s0 = 16 * i + j0
                                    cp_fn = (nc.vector.tensor_copy,
                                             nc.scalar.copy)[cp_rr[0] % 2]
                                    cp_rr[0] += 1
                                    cp_fn(
                                        m04[:, :, cb:cb + nb,
                                            s0:s0 + jn].rearrange(
                                                "p t c s -> p s c t"),
                                        pm[:, :jn * w_].rearrange(
                                            "p (s c t) -> p s c t",
                                            c=nb, t=T))

                # state init
                nc.vector.tensor_copy(
                    qw_t[:, :, :],
                    xsq_sb[:, None, :].broadcast_to((128, CP, S)))
                nc.gpsimd.memset(A_t[:], 1.0)
                nc.gpsimd.memset(Q_t[:], 1.0)

                # =======================  flow phase  =======================
                # Lookahead correction: during step t we precompute
                #   corrpre(t+1) = sum_{k<t} u_k * G[k, t+1]
                # (bulk multiply + bf16 add-tree, off the serial chain) and
                #   s1(t+1) = m0[t+1] - corrpre(t+1).
                # The inter-step chain then only carries the last column:
                #   d1(t+1) = s1(t+1) - u_t * G[t, t+1].
                # The r^2 / qw / Q scalar chains run on the GPSIMD engine.
                gstate = [dict() for _ in GROUPS]

                def gslice(g):
                    c0, G_ = GROUPS[g]
                    return (slice(None), slice(c0, c0 + G_), slice(None))

                def flow_head(g, t):
                    c0, G_ = GROUPS[g]
                    st = gstate[g]
                    FGS = G_ * S
                    d1 = d1_t[gslice(g)]
                    m0col = m04[:, t, c0:c0 + G_, :]
                    u4 = uacc4[:, c0:c0 + G_, :, :]

                    def tl(name):
                        return sc.tile([128, G_, S], F, name=f"{name}{g}")

                    # ---- corr = sum_{k<t} u_k * G[k, t]  (lazy dot) ----
                    if t == 0:
                        nc.vector.tensor_copy(d1, m0col)
                    else:
                        w4 = wtl[:, :FGS * t].rearrange(
                            "p (c s k) -> p c s k", c=G_, k=t)
                        gview = gb4[:, c0:c0 + G_, t, 0:t][:, :, None, :]
                        nc.vector.tensor_tensor(
                            out=w4, in0=u4[:, :, :, 0:t],
                            in1=gview.broadcast_to((128, G_, S, t)), op=OP.mult)
                        k = t
                        if k > 2:
                            p2 = 1 << (k.bit_length() - 1)
                            if p2 == k:
                                p2 //= 2
                            nc.vector.tensor_tensor(
                                out=w4[:, :, :, 0:k - p2],
                                in0=w4[:, :, :, 0:k - p2],
                                in1=w4[:, :, :, p2:k], op=OP.add)
                            k = p2
                            while k > 2:
                                h = k // 2
                                nc.vector.tensor_tensor(
                                    out=w4[:, :, :, 0:h], in0=w4[:, :, :, 0:h],
                                    in1=w4[:, :, :, h:k], op=OP.add)
                                k = h
                        nc.vector.tensor_tensor(
                            out=d1, in0=m0col, in1=w4[:, :, :, 0],
                            op=OP.subtract)
                        if k == 2:
                            nc.vector.tensor_tensor(
                                out=d1, in0=d1, in1=w4[:, :, :, 1],
                                op=OP.subtract)
                    dd = tl("dd")
                    de = nc.vector if t < TEQ else nc.gpsimd
                    de.tensor_tensor(out=dd[:], in0=d1, in1=d1, op=OP.add)
                    st["dd"] = dd

                def flow_mid(g, t):
                    nonlocal last_flow_act
                    c0, G_ = GROUPS[g]
                    st = gstate[g]
                    dd = st["dd"]
                    sl3 = gslice(g)
                    qw = qw_t[sl3]
                    A = A_t[sl3]
                    Q = Q_t[sl3]
                    d1 = d1_t[sl3]
                    rs = rs_t[sl3]
                    bh = bh_t[sl3]
                    rs2 = rs2_t[sl3]
                    u4 = uacc4[:, c0:c0 + G_, :, :]

                    def tl(name):
                        return sc.tile([128, G_, S], F, name=f"{name}{g}")

                    # ---- r2 = A*(A*qw - 2*d1) + n0sq ----
                    g1 = tl("g1")
                    nc.vector.tensor_tensor(out=g1[:], in0=A, in1=qw,
                                            op=OP.mult)
                    g2 = tl("g2")
                    nc.vector.scalar_tensor_tensor(g2[:], d1, -2.0, g1[:],
                                                   op0=OP.mult, op1=OP.add)
                    r2m = tl("g1")      # g1 dead after g2
                    nc.vector.tensor_tensor(out=r2m[:], in0=A, in1=g2[:],
                                            op=OP.mult)
                    r = tl("lc")        # lc dead after d1
                    for ci in range(G_):
                        ct = T * (c0 + ci) + t
                        last_flow_act = nc.scalar.activation(
                            r[:, ci, :], r2m[:, ci, :], AF.Sqrt,
                            bias=n0sq_sb[:, ct:ct + 1], scale=1.0)
                    # s = r + alpha ; rs = 1/s ; bh = beta*rs
                    # early steps are chain-bound: keep s_t on DVE there
                    s_t = tl("s_t")
                    if t < TES:
                        av = alpha_sb.rearrange("p (c t) -> p c t", t=T)[
                            :, c0:c0 + G_, t][:, :, None]
                        nc.vector.tensor_tensor(
                            out=s_t[:], in0=r[:],
                            in1=av.broadcast_to((128, G_, S)), op=OP.add)
                    else:
                        for ci in range(G_):
                            ct = T * (c0 + ci) + t
                            nc.scalar.activation(
                                s_t[:, ci, :], r[:, ci, :], AF.Identity,
                                bias=alpha_sb[:, ct:ct + 1], scale=1.0)
                    nc.vector.reciprocal_approx_fast(rs, s_t[:])
                    bv = beta_sb.rearrange("p (c t) -> p c t", t=T)[
                        :, c0:c0 + G_, t][:, :, None]
                    nc.vector.tensor_tensor(
                        out=bh, in0=rs,
                        in1=bv.broadcast_to((128, G_, S)), op=OP.mult)
                    # Q *= 1 + ab*rs^2   (scalar engine square, gpsimd chain)
                    nc.scalar.activation(rs2, rs, AF.Square)
                    k1 = tl("s_t")      # s_t dead after rs
                    abv = ab_sb.rearrange("p (c t) -> p c t", t=T)[
                        :, c0:c0 + G_, t][:, :, None]
                    nc.gpsimd.tensor_tensor(
                        out=k1[:], in0=rs2,
                        in1=abv.broadcast_to((128, G_, S)), op=OP.mult)
                    # Q *= (1 + v)  as  Q += Q*v  (no scalar ops on Pool)
                    k2 = tl("g1")
                    nc.gpsimd.tensor_tensor(out=k2[:], in0=Q, in1=k1[:],
                                            op=OP.mult)
                    nc.gpsimd.tensor_tensor(out=Q, in0=Q, in1=k2[:], op=OP.add)
                    # A' = (1+bh)*A  (in place)
                    nc.vector.scalar_tensor_tensor(A, bh, 1.0, A,
                                                   op0=OP.add, op1=OP.mult)
                    # ut = bh / A'  (stored bf16 into the u history)
                    rA = tl("g2")       # g2 dead after r2m
                    nc.vector.reciprocal_approx_fast(rA[:], A)
                    ut = u4[:, :, :, t]
                    nc.vector.tensor_tensor(out=ut, in0=bh, in1=rA[:],
                                            op=OP.mult)
                    # qw' = qw + ut*(ut*Gtt - 2*d1)
                    # (gpsimd once steps are long enough to hide it)
                    qe = nc.gpsimd
                    gttv = gb4[:, c0:c0 + G_, t, t][:, :, None]
                    h1 = tl("h1")
                    qe.tensor_tensor(
                        out=h1[:], in0=ut,
                        in1=gttv.broadcast_to((128, G_, S)), op=OP.mult)
                    h2 = tl("h2")
                    qe.tensor_tensor(out=h2[:], in0=h1[:], in1=dd[:],
                                     op=OP.subtract)
                    h3 = tl("h1")       # h1 dead after h2
                    qe.tensor_tensor(out=h3[:], in0=ut, in1=h2[:],
                                     op=OP.mult)
                    qe.tensor_tensor(out=qw, in0=qw, in1=h3[:],
                                     op=OP.add)

                for t in range(T):
                    for g in range(len(GROUPS)):
                        flow_head(g, t)
                    for g in range(len(GROUPS)):
                        flow_mid(g, t)

            # =========================  epilogue  =========================
            # Pin all epilogue ACT work behind a single natural_log_exp table
            # load (Sqrt/Ln/Exp live in different sets).
            nle_id = list(get_activation_tables(nc.m.arch)).index(
                "natural_log_exp_and_others")
            tbl_load = mybir.InstLoadActFuncSet(
                name=f"I-{nc.next_id()}", act_func_set_id=nle_id, ins=[], outs=[])
            tl_bi = nc.scalar.add_instruction(tbl_load)
            add_dep_helper(tl_bi.ins, last_flow_act.ins, True,
                           "table load after flow phase")

            def act_pinned(out, in_, func, **kw):
                bi = nc.scalar.activation(out, in_, func, **kw)
                add_dep_helper(bi.ins, tl_bi.ins, True, "epilogue act after load")
                return bi

            with tc.tile_pool(name="epi", bufs=1) as ep:
                lpw = ep.tile([128, CP * S], F)
                lpw3 = lpw.rearrange("p (c s) -> p c s", s=S)
                # lpw = -0.5*A^2*qw + 63*ln(A) + ln(Q) + cadd
                # za/zq/zqc only need flow state -> gpsimd, ahead of the Lns
                za = ep.tile([128, CP, S], F)
                nc.vector.tensor_tensor(out=za[:, :, :], in0=A_t[:, :, :],
                                        in1=A_t[:, :, :], op=OP.mult)
                zq = ep.tile([128, CP, S], F)
                nc.vector.tensor_tensor(out=zq[:, :, :], in0=za[:, :, :],
                                        in1=qw_t[:, :, :], op=OP.mult)
                cv = cadd_sb[:, :, None]
                zqc = ep.tile([128, CP, S], F)
                nc.vector.scalar_tensor_tensor(
                    zqc[:, :, :], zq[:, :, :], -0.5,
                    cv.broadcast_to((128, CP, S)),
                    op0=OP.mult, op1=OP.add)
                l1 = ep.tile([128, CP * S], F)
                act_pinned(l1[:], A_t[:, :, :].rearrange("p c s -> p (c s)"),
                           AF.Ln)
                l2 = ep.tile([128, CP * S], F)
                act_pinned(l2[:], Q_t[:, :, :].rearrange("p c s -> p (c s)"),
                           AF.Ln)
                w1 = ep.tile([128, CP * S], F)
                nc.vector.scalar_tensor_tensor(w1[:], l1[:], float(D - 1), l2[:],
                                               op0=OP.mult, op1=OP.add)
                nc.vector.tensor_tensor(
                    out=lpw3, in0=w1.rearrange("p (c s) -> p c s", s=S),
                    in1=zqc[:, :, :], op=OP.add)

                lpw_perm = lpw.rearrange("p (c s) -> p s c", s=S)
                mscl = ep.tile([128, 3, S], F)
                mx = mscl[:, 0, :]
                se = mscl[:, 1, :]
                clsl = mscl[:, 2, :]
                nc.vector.tensor_reduce(mx, lpw_perm, axis=AX.X, op=OP.max)
                exs = ep.tile([128, CP * S], F)
                exs3 = exs.rearrange("p (c s) -> p c s", s=S)
                mx_b = mx[:, None, :].broadcast_to((128, CP, S))
                nc.vector.tensor_tensor(out=exs3, in0=lpw3[:, :, :], in1=mx_b,
                                        op=OP.subtract)
                act_pinned(exs[:], exs[:], AF.Exp)
                nc.vector.tensor_reduce(
                    se, exs.rearrange("p (c s) -> p s c", s=S),
                    axis=AX.X, op=OP.add)
                msk_sb = ep.tile([128, CP * S], F)
                nc.sync.dma_start(msk_sb[:], masksb[:])
                gsum = ep.tile([128, CP * S], F)   # own buffer: the mask
                # path runs in parallel with the sum-exp path
                nc.vector.tensor_tensor(out=gsum[:], in0=msk_sb[:], in1=lpw[:],
                                        op=OP.mult)
                nc.vector.tensor_reduce(
                    clsl, gsum.rearrange("p (c s) -> p s c", s=S),
                    axis=AX.X, op=OP.add)

                # ---- AllToAll: ccin[j] = (mx, se, cls) for sample-slice j ----
                ccin = dp.tile([NCORES, 3, 128 * SL], F)
                ccout = dp.tile([NCORES, 3, 128 * SL], F)
                ccin_v = ccin.rearrange("r t (p s) -> t p r s", p=128)
                for ti in range(3):
                    nc.sync.dma_start(
                        ccin_v[ti],
                        mscl[:, ti, :].rearrange("p (r s) -> p r s", s=SL))
                nc.gpsimd.collective_compute(
                    "AllToAll", OP.bypass,
                    replica_groups=[list(range(NCORES))],
                    ins=[ccin.opt()], outs=[ccout.opt()],
                )
                # ---- logits path fills the AllToAll wait ----
                xsl_sb = ep.tile([D + 1, 128 * SL], F)
                nc.sync.dma_start(xsl_sb[:], xslice[:])
                Wb_sb = ep.tile([D + 1, C], F)
                nc.sync.dma_start(Wb_sb[:], Wb[:])
                lg = ep.tile([128, SL * C], F)
                for j in range(SL):
                    pl = plp.tile([128, C], F)
                    nc.tensor.matmul(pl[:],
                                     lhsT=xsl_sb[:, 128 * j:128 * (j + 1)],
                                     rhs=Wb_sb[:], start=True, stop=True)
                    nc.scalar.copy(lg[:, C * j:C * (j + 1)], pl[:])
                lg3 = lg.rearrange("p (s c) -> p s c", c=C)
                ml = ep.tile([128, SL], F)
                nc.vector.tensor_reduce(ml[:], lg3, axis=AX.X, op=OP.max)
                ml_b = ml[:, :, None].broadcast_to((128, SL, C))
                nc.vector.tensor_tensor(out=lg3, in0=lg3, in1=ml_b,
                                        op=OP.subtract)
                act_pinned(lg[:], lg[:], AF.Exp)
                ssum = ep.tile([128, SL], F)
                nc.vector.tensor_reduce(ssum[:], lg3, axis=AX.X, op=OP.add)
                rsum = ep.tile([128, SL], F)
                rscr = ep.tile([128, SL], F)
                nc.vector.reciprocal_approx_accurate(rsum[:], ssum[:], rscr[:])

                ccout_v = ccout.rearrange("r t (p s) -> t p r s", p=128)
                cco = ep.tile([128, 3, NCORES, SL], F)
                for ti in range(3):
                    nc.sync.dma_start(cco[:, ti], ccout_v[ti])
                mxg = cco[:, 0]
                seg = cco[:, 1]
                clg = cco[:, 2]

                # ---- global combine for our slice ----
                M = ep.tile([128, SL], F)
                nc.vector.tensor_reduce(M[:], mxg.rearrange("p r s -> p s r"),
                                        axis=AX.X, op=OP.max)
                esh = ep.tile([128, NCORES * SL], F)
                esh3 = esh.rearrange("p (r s) -> p r s", s=SL)
                M_b = M[:, None, :].broadcast_to((128, NCORES, SL))
                nc.vector.tensor_tensor(out=esh3, in0=mxg, in1=M_b,
                                        op=OP.subtract)
                act_pinned(esh[:], esh[:], AF.Exp)
                wsum = ep.tile([128, NCORES * SL], F)
                nc.vector.tensor_tensor(out=wsum[:], in0=esh[:], in1=seg.rearrange("p r s -> p (r s)"),
                                        op=OP.mult)
                Sg = ep.tile([128, SL], F)
                nc.vector.tensor_reduce(
                    Sg[:], wsum.rearrange("p (r s) -> p s r", s=SL),
                    axis=AX.X, op=OP.add)
                lse = ep.tile([128, SL], F)
                act_pinned(lse[:], Sg[:], AF.Ln)
                nc.vector.tensor_tensor(out=lse[:], in0=lse[:], in1=M[:],
                                        op=OP.add)
                clsf = ep.tile([128, SL], F)
                nc.vector.tensor_reduce(clsf[:], clg.rearrange("p r s -> p s r"),
                                        axis=AX.X, op=OP.add)
                corr_sb = ep.tile([128, SL], F)
                nc.sync.dma_start(corr_sb[:], corr_in[:])
                nc.vector.tensor_tensor(out=clsf[:], in0=clsf[:], in1=corr_sb[:],
                                        op=OP.subtract)
                lev = ep.tile([128, SL], F)
                nc.vector.tensor_scalar(out=lev[:], in0=lse[:],
                                        scalar1=EV_BUDGET,
                                        scalar2=LOG_EV_CLAMP, op0=OP.add,
                                        op1=OP.min)
                ev = ep.tile([128, SL], F)
                act_pinned(ev[:], lev[:], AF.Exp)

                # ---- combine evidence with precomputed softmax ----
                evn = ep.tile([128, SL], F)
                nc.vector.tensor_tensor(out=evn[:], in0=ev[:], in1=rsum[:],
                                        op=OP.mult)
                evn_b = evn[:, :, None].broadcast_to((128, SL, C))
                t1 = lg  # in-place: exp(logits) no longer needed afterwards
                t13 = lg3
                nc.vector.tensor_tensor(out=t13, in0=lg3, in1=evn_b, op=OP.mult)
                la = gsum[:, :SL * C]  # gsum dead after the cls reduce
                act_pinned(la[:], t1[:], AF.Ln, bias=1.0)
                # accurate log1p for small x: x*(1 + x*(-1/2 + x/3)) when x<0.01
                h1e = ep.tile([128, SL * C], F)
                nc.vector.tensor_scalar(out=h1e[:], in0=t1[:], scalar1=1.0 / 3.0,
                                        scalar2=-0.5, op0=OP.mult, op1=OP.add)
                nc.vector.tensor_tensor(out=h1e[:], in0=h1e[:], in1=t1[:],
                                        op=OP.mult)
                nc.vector.tensor_scalar_add(h1e[:], h1e[:], 1.0)
                nc.vector.tensor_tensor(out=h1e[:], in0=h1e[:], in1=t1[:],
                                        op=OP.mult)
                h2e = h1e
                lmask = ep.tile([128, SL * C], mybir.dt.uint8)
                nc.vector.tensor_scalar(out=lmask[:], in0=t1[:], scalar1=0.01,
                                        scalar2=None, op0=OP.is_lt)
                nc.vector.select(la[:], lmask[:], h2e[:], la[:])

                ob = lpw[:, :SL * (C + 1)]  # lpw dead after gsum
                ob3 = ob.rearrange("p (s c) -> p s c", c=C + 1)
                la3 = la.rearrange("p (s c) -> p s c", c=C)
                out_v = out_d.rearrange("(s p) c -> p s c", p=128)
                H = SL // 2
                nc.vector.tensor_copy(ob3[:, :H, 0:C], la3[:, :H, :])
                nc.vector.tensor_copy(ob3[:, :H, C:C + 1], clsf[:, :H, None])
                nc.sync.dma_start(out_v[:, :H], ob3[:, :H, :])
                nc.vector.tensor_copy(ob3[:, H:, 0:C], la3[:, H:, :])
                nc.vector.tensor_copy(ob3[:, H:, C:C + 1], clsf[:, H:, None])
                nc.sync.dma_start(out_v[:, H:], ob3[:, H:, :])

    nc.finalize()
    return nc


def _softplus(v):
    return np.log1p(np.exp(-np.abs(v))) + np.maximum(v, 0)


def host_prep(x, labels, labels_frequency, z0, alpha_prime, beta_prime, W, b):
    import ml_dtypes
    x = np.asarray(x, np.float32)
    labels = np.asarray(labels).astype(np.int64)
    freq = np.asarray(labels_frequency, np.float32)
    z0 = np.asarray(z0, np.float32)
    alpha = _softplus(np.asarray(alpha_prime, np.float32)).astype(np.float32)
    beta = (-alpha + _softplus(np.asarray(beta_prime, np.float32))).astype(np.float32)
    W = np.asarray(W, np.float32)
    b = np.asarray(b, np.float32)

    xaugT = np.concatenate([x.T, np.ones((1, N), np.float32)], axis=0)  # [65, N]
    xbf = np.ascontiguousarray(x.T).astype(ml_dtypes.bfloat16)          # [D, N]
    Wb = np.concatenate([W, b[None, :]], axis=0).astype(np.float32)    # [65, C]
    xsq = np.sum(x * x, axis=1).astype(np.float32).reshape(S, 128).T   # [128, S]
    logfreq = np.log(freq).astype(np.float32)
    lab_ps = labels.reshape(S, 128).T                                  # [128, S]

    ones128 = np.ones((128, 1), np.float32)
    in_maps = []
    for k, (cls, real) in enumerate(_class_split()):
        z0c = z0[cls]                                   # [CP, T, D]
        alc = alpha[cls]                                # [CP, T]
        bec = beta[cls]
        G = np.einsum('cij,ckj->cik', z0c, z0c).astype(np.float32)   # [CP,T,T]
        n0 = np.sum(z0c * z0c, axis=2).astype(np.float32)            # [CP, T]
        Gb = np.broadcast_to(
            G.astype(ml_dtypes.bfloat16).reshape(CP, 1, T * T),
            (CP, 128, T * T)).copy()
        ab = (alc * bec).astype(np.float32)
        tabs = np.concatenate([alc.reshape(-1), bec.reshape(-1),
                               n0.reshape(-1), ab.reshape(-1)])
        tabs_rk = np.broadcast_to(tabs.reshape(1, 4 * CP * T),
                                  (128, 4 * CP * T)).copy()
        cadd = np.array([(logfreq[c] + NEG_HALF_DLOG2PI) if r else PAD_NEGINF
                         for c, r in zip(cls, real)], np.float32)
        cadd_rk = (ones128 * cadd[None, :]).astype(np.float32)
        msk = np.zeros((128, CP, S), np.float32)
        for i, (c, r) in enumerate(zip(cls, real)):
            if r:
                msk[:, i, :] = (lab_ps == c)
        sl = slice(1024 * k, 1024 * (k + 1))
        corr_k = logfreq[labels[sl]].reshape(SL, 128).T.astype(np.float32)
        in_maps.append(dict(
            xbf=xbf, xslice=np.ascontiguousarray(xaugT[:, sl]), Wb=Wb,
            xsq=xsq,
            z0T=np.ascontiguousarray(z0c.transpose(2, 0, 1)).reshape(
                D, CP * T).astype(ml_dtypes.bfloat16),
            Gb=Gb, tabs_r=tabs_rk,
            cadd_r=cadd_rk, corr=corr_k,
            masksb=msk.reshape(128, CP * S),
        ))
    return in_maps


def kernel(**inputs) -> np.ndarray:
    if "nc" not in _CACHE:
        _CACHE["nc"] = build_program()
    nc = _CACHE["nc"]
    in_maps = host_prep(**inputs)
    if os.environ.get("KERNEL_SIM"):
        from concourse.bass_interp import MultiCoreSim
        sim = MultiCoreSim(nc, NCORES)
        for k in range(NCORES):
            for name, arr in in_maps[k].items():
                sim.cores[k].tensor(name)[:] = arr
        sim.simulate()
        outs = [np.array(sim.cores[k].tensor("out")) for k in range(NCORES)]
    else:
        res = run_bass_kernel_spmd(nc, in_maps, list(range(NCORES)))
        outs = [res.results[k]["out"] for k in range(NCORES)]
    return np.concatenate(outs, axis=0)


# revision 90
# speedup vs baseline: 1.0043x; 1.0018x over previous
"""NatPN radial-flow posterior kernel for Trainium2, 8 NeuronCores (SPMD).

Strategy (class-sharded "expert parallel", lazy-correction flow):
  * Radial flow z' = z + beta*h*(z - z0), h = 1/(alpha + r), r = |z - z0| is
    run in coefficient space z_t = A_t * w_t, w_t = x - sum_k u_k z0_k.
    Per step we need only the scalar dot d1 = w . z0_t, recovered LAZILY:
        d1_t = m0[t] - sum_{k<t} u_k G[k, t]
    with m0 = x @ z0^T (tensor engine) and G = z0 @ z0^T (host). The inner
    product runs as one bf16 2x-mode multiply (u-history and G rows both have
    packed last dims) plus a bf16 add-tree + small tensor_reduce — about half
    the vector-engine cost of eagerly maintaining the m table.
  * State per (class, sample): qw = |w|^2, A = prod(1+bh) (which IS the P
    determinant product), Q = prod(1+alpha*beta*h^2). The qw/Q updates run on
    the otherwise-idle GPSIMD engine; r = sqrt(...) on the scalar engine.
  * 8 cores x 13 classes (100 real + 4 padded with freq -> -1e30), classes
    split into two pipelined groups (7+6) so engines overlap.
  * Epilogue: local max / sum-exp / label-masked partials, one AllToAll
    (96 KB), then each core finishes logsumexp + softmax + Dirichlet update
    for its own 1024-sample slice and writes rows [1024k, 1024(k+1)).
"""
import os
import numpy as np

import concourse.bass as bass
import concourse.bacc as bacc
import concourse.mybir as mybir
from concourse import tile
from concourse.bass_utils import run_bass_kernel_spmd
from concourse.hw_specs import get_activation_tables
from concourse.tile_rust import add_dep_helper

F = mybir.dt.float32
BF = mybir.dt.bfloat16
AF = mybir.ActivationFunctionType
OP = mybir.AluOpType
AX = mybir.AxisListType

NCORES = 8
N, D, C, T = 8192, 64, 100, 30
CP = 13            # classes per core (padded)
S = 64             # sample groups of 128 (N = 128 * S)
SL = 8             # sample groups per core in the epilogue slice
LOG_EV_CLAMP = 10.0
EV_BUDGET = 0.5 * D * float(np.log(4.0 * np.pi))
NEG_HALF_DLOG2PI = -0.5 * D * float(np.log(2.0 * np.pi))
TES = 8            # s_t on DVE below this step
TEQ = 8            # qw/dd chain on DVE below this step
PAD_NEGINF = -1.0e30

_CACHE = {}


def _class_split():
    """cores 0-3 get 13 real classes, cores 4-7 get 12 real + 1 pad."""
    out = []
    off = 0
    for k in range(NCORES):
        cnt = 13 if k < 4 else 12
        cls = list(range(off, off + cnt))
        off += cnt
        real = [True] * cnt
        while len(cls) < CP:
            cls.append(0)
            real.append(False)
        out.append((cls, real))
    assert off == C
    return out


def build_program():
    nc = bacc.Bacc("TRN2", target_bir_lowering=False, debug=False,
                   num_devices=NCORES)

    xbf = nc.dram_tensor("xbf", [D, N], BF, kind="ExternalInput")
    xslice = nc.dram_tensor("xslice", [D + 1, 128 * SL], F, kind="ExternalInput")
    Wb = nc.dram_tensor("Wb", [D + 1, C], F, kind="ExternalInput")
    xsq = nc.dram_tensor("xsq", [128, S], F, kind="ExternalInput")
    z0T = nc.dram_tensor("z0T", [D, CP * T], BF, kind="ExternalInput")
    Gb = nc.dram_tensor("Gb", [CP, 128, T * T], BF, kind="ExternalInput")
    # alpha | beta | n0sq | ab stacked into one tensor -> one DMA
    tabs_r = nc.dram_tensor("tabs_r", [128, 4 * CP * T], F, kind="ExternalInput")
    cadd_r = nc.dram_tensor("cadd_r", [128, CP], F, kind="ExternalInput")
    corr_in = nc.dram_tensor("corr", [128, SL], F, kind="ExternalInput")
    masksb = nc.dram_tensor("masksb", [128, CP * S], F, kind="ExternalInput")
    out_d = nc.dram_tensor("out", [128 * SL, C + 1], F, kind="ExternalOutput")

    GROUPS = [(0, 7), (7, 6)]   # (class offset, count)

    with tile.TileContext(nc) as tc:
        with tc.tile_pool(name="const", bufs=1) as cp_, \
             tc.tile_pool(name="st", bufs=1) as stp, \
             tc.tile_pool(name="pm", bufs=6, space="PSUM") as pmp, \
             tc.tile_pool(name="pl", bufs=2, space="PSUM") as plp, \
             tc.tile_pool(name="dram", bufs=1, space="DRAM") as dp:

            # ---- resident constants / state ----
            # (DMA issue order matters: x chunk + z0 first so the tensor
            #  engine starts immediately; big Gb load afterwards)
            tabs_sb = cp_.tile([128, 4 * CP * T], F)
            CT = CP * T
            alpha_sb = tabs_sb[:, 0:CT]
            beta_sb = tabs_sb[:, CT:2 * CT]
            n0sq_sb = tabs_sb[:, 2 * CT:3 * CT]
            ab_sb = tabs_sb[:, 3 * CT:4 * CT]
            xsq_sb = cp_.tile([128, S], F)
            cadd_sb = cp_.tile([128, CP], F)
            gb_sb = cp_.tile([128, CP * T * T], BF)
            gb4 = gb_sb.rearrange("p (c t j) -> p c t j", c=CP, j=T)

            # per-(c,s) f32 state, CP-wide (groups use class sub-ranges)
            qw_t = stp.tile([128, CP, S], F)
            A_t = stp.tile([128, CP, S], F)
            Q_t = stp.tile([128, CP, S], F)
            d1_t = stp.tile([128, CP, S], F)
            rs_t = stp.tile([128, CP, S], F)
            bh_t = stp.tile([128, CP, S], F)
            rs2_t = stp.tile([128, CP, S], F)

            last_flow_act = None

            with tc.tile_pool(name="big", bufs=1) as bigp, \
                 tc.tile_pool(name="sc", bufs=1) as sc:
                m0 = bigp.tile([128, T * CP * S], BF)        # t-major: [t, c, s]
                m04 = m0.rearrange("p (t c s) -> p t c s", t=T, s=S)
                uacc = bigp.tile([128, CP * S * T], BF)      # k-minor: [c, s, k]
                uacc4 = uacc.rearrange("p (c s k) -> p c s k", s=S, k=T)
                wtl = bigp.tile([128, 7 * S * (T - 1)], BF)  # shared scratch

                # ---- init: m0 = x @ z0^T on the tensor engine ----
                # x loaded in 2048-sample chunks; group A's classes complete
                # first so its flow steps start while group B still inits.
                with tc.tile_pool(name="init", bufs=1) as ip:
                    xch0 = ip.tile([D, 128 * 16], BF, name="xch0")
                    nc.sync.dma_start(xch0[:], xbf[:, 0:2048])
                    z0all = ip.tile([D, CP * T], BF)
                    nc.sync.dma_start(z0all[:], z0T[:])
                    # remaining constants (needed only once flow starts)
                    nc.sync.dma_start(tabs_sb[:], tabs_r[:])
                    nc.sync.dma_start(xsq_sb[:], xsq[:])
                    nc.sync.dma_start(cadd_sb[:], cadd_r[:])
                    nc.sync.dma_start(
                        gb_sb[:].rearrange("p (c g) -> p c g", c=CP),
                        Gb[:].rearrange("c p g -> p c g"))
                    cp_rr = [0]
                    for (c0g, Gg) in GROUPS:
                        # class blocks of up to 4 share one rhs (wider
                        # matmuls -> 3.5x fewer PE instructions)
                        blocks = []
                        cc = c0g
                        while cc < c0g + Gg:
                            nb = min(4, c0g + Gg - cc)
                            blocks.append((cc, nb))
                            cc += nb
                        for i in range(S // 16):
                            if (c0g, i) == (0, 0):
                                xch = xch0
                            else:
                                xch = ip.tile([D, 128 * 16], BF,
                                              name=f"xch{i % 2}")
                                nc.sync.dma_start(
                                    xch[:], xbf[:, 2048 * i:2048 * (i + 1)])
                            for (cb, nb) in blocks:
                                w_ = T * nb          # rhs cols
                                jm = 480 // w_       # samples per PSUM tile
                                for j0 in range(0, 16, jm):
                                    jn = min(jm, 16 - j0)
                                    pm = pmp.tile([128, jm * w_], F, name="pm")
                                    for j in range(jn):
                                        s_ = 128 * (j0 + j)
                                        nc.tensor.matmul(
                                            pm[:, w_ * j:w_ * (j + 1)],
                                            lhsT=xch[:, s_:s_ + 128],
                                            rhs=z0all[:, T * cb:T * cb + w_],
                                            start=True, stop=True)
                                    # pm dims (j, c, t) -> m0[t, c, 16i+j0+j]
                                    # copies round-robin across DVE/ACT/Pool
                                    # (all three idle during init)
                                    s0 = 16 * i + j0
                                    cp_fn = (nc.vector.tensor_copy,
                                             nc.scalar.copy)[cp_rr[0] % 2]
                                    cp_rr[0] += 1
                                    cp_fn(
                                        m04[:, :, cb:cb + nb,
                                            s0:s0 + jn].rearrange(
                                                "p t c s -> p s c t"),
                                        pm[:, :jn * w_].rearrange(
                                            "p (s c t) -> p s c t",
                                            c=nb, t=T))

                # state init
                nc.vector.tensor_copy(
                    qw_t[:, :, :],
                    xsq_sb[:, None, :].broadcast_to((128, CP, S)))
                nc.gpsimd.memset(A_t[:], 1.0)
                nc.gpsimd.memset(Q_t[:], 1.0)

                # =======================  flow phase  =======================
                # Lookahead correction: during step t we precompute
                #   corrpre(t+1) = sum_{k<t} u_k * G[k, t+1]
                # (bulk multiply + bf16 add-tree, off the serial chain) and
                #   s1(t+1) = m0[t+1] - corrpre(t+1).
                # The inter-step chain then only carries the last column:
                #   d1(t+1) = s1(t+1) - u_t * G[t, t+1].
                # The r^2 / qw / Q scalar chains run on the GPSIMD engine.
                gstate = [dict() for _ in GROUPS]

                def gslice(g):
                    c0, G_ = GROUPS[g]
                    return (slice(None), slice(c0, c0 + G_), slice(None))

                def flow_head(g, t):
                    c0, G_ = GROUPS[g]
                    st = gstate[g]
                    FGS = G_ * S
                    d1 = d1_t[gslice(g)]
                    m0col = m04[:, t, c0:c0 + G_, :]
                    u4 = uacc4[:, c0:c0 + G_, :, :]

                    def tl(name):
                        return sc.tile([128, G_, S], F, name=f"{name}{g}")

                    # ---- corr = sum_{k<t} u_k * G[k, t]  (lazy dot) ----
                    if t == 0:
                        nc.vector.tensor_copy(d1, m0col)
                    else:
                        w4 = wtl[:, :FGS * t].rearrange(
                            "p (c s k) -> p c s k", c=G_, k=t)
                        gview = gb4[:, c0:c0 + G_, t, 0:t][:, :, None, :]
                        nc.vector.tensor_tensor(
                            out=w4, in0=u4[:, :, :, 0:t],
                            in1=gview.broadcast_to((128, G_, S, t)), op=OP.mult)
                        k = t
                        if k > 2:
                            p2 = 1 << (k.bit_length() - 1)
                            if p2 == k:
                                p2 //= 2
                            nc.vector.tensor_tensor(
                                out=w4[:, :, :, 0:k - p2],
                                in0=w4[:, :, :, 0:k - p2],
                                in1=w4[:, :, :, p2:k], op=OP.add)
                            k = p2
                            while k > 2:
                                h = k // 2
                                nc.vector.tensor_tensor(
                                    out=w4[:, :, :, 0:h], in0=w4[:, :, :, 0:h],
                                    in1=w4[:, :, :, h:k], op=OP.add)
                                k = h
                        nc.vector.tensor_tensor(
                            out=d1, in0=m0col, in1=w4[:, :, :, 0],
                            op=OP.subtract)
                        if k == 2:
                            nc.vector.tensor_tensor(
                                out=d1, in0=d1, in1=w4[:, :, :, 1],
                                op=OP.subtract)
                    dd = tl("dd")
                    de = nc.vector if t < TEQ else nc.gpsimd
                    de.tensor_tensor(out=dd[:], in0=d1, in1=d1, op=OP.add)
                    st["dd"] = dd

                def flow_mid(g, t):
                    nonlocal last_flow_act
                    c0, G_ = GROUPS[g]
                    st = gstate[g]
                    dd = st["dd"]
                    sl3 = gslice(g)
                    qw = qw_t[sl3]
                    A = A_t[sl3]
                    Q = Q_t[sl3]
                    d1 = d1_t[sl3]
                    rs = rs_t[sl3]
                    bh = bh_t[sl3]
                    rs2 = rs2_t[sl3]
                    u4 = uacc4[:, c0:c0 + G_, :, :]

                    def tl(name):
                        return sc.tile([128, G_, S], F, name=f"{name}{g}")

                    # ---- r2 = A*(A*qw - 2*d1) + n0sq ----
                    g1 = tl("g1")
                    nc.vector.tensor_tensor(out=g1[:], in0=A, in1=qw,
                                            op=OP.mult)
                    g2 = tl("g2")
                    nc.vector.scalar_tensor_tensor(g2[:], d1, -2.0, g1[:],
                                                   op0=OP.mult, op1=OP.add)
                    r2m = tl("g1")      # g1 dead after g2
                    nc.vector.tensor_tensor(out=r2m[:], in0=A, in1=g2[:],
                                            op=OP.mult)
                    r = tl("lc")        # lc dead after d1
                    for ci in range(G_):
                        ct = T * (c0 + ci) + t
                        last_flow_act = nc.scalar.activation(
                            r[:, ci, :], r2m[:, ci, :], AF.Sqrt,
                            bias=n0sq_sb[:, ct:ct + 1], scale=1.0)
                    # s = r + alpha ; rs = 1/s ; bh = beta*rs
                    # early steps are chain-bound: keep s_t on DVE there
                    s_t = tl("s_t")
                    if t < TES:
                        av = alpha_sb.rearrange("p (c t) -> p c t", t=T)[
                            :, c0:c0 + G_, t][:, :, None]
                        nc.vector.tensor_tensor(
                            out=s_t[:], in0=r[:],
                            in1=av.broadcast_to((128, G_, S)), op=OP.add)
                    else:
                        for ci in range(G_):
                            ct = T * (c0 + ci) + t
                            nc.scalar.activation(
                                s_t[:, ci, :], r[:, ci, :], AF.Identity,
                                bias=alpha_sb[:, ct:ct + 1], scale=1.0)
                    nc.vector.reciprocal_approx_fast(rs, s_t[:])
                    bv = beta_sb.rearrange("p (c t) -> p c t", t=T)[
                        :, c0:c0 + G_, t][:, :, None]
                    nc.vector.tensor_tensor(
                        out=bh, in0=rs,
                        in1=bv.broadcast_to((128, G_, S)), op=OP.mult)
                    # Q *= 1 + ab*rs^2   (scalar engine square, gpsimd chain)
                    nc.scalar.activation(rs2, rs, AF.Square)
                    k1 = tl("s_t")      # s_t dead after rs
                    abv = ab_sb.rearrange("p (c t) -> p c t", t=T)[
                        :, c0:c0 + G_, t][:, :, None]
                    pe_ = nc.gpsimd
                    pe_.tensor_tensor(
                        out=k1[:], in0=rs2,
                        in1=abv.broadcast_to((128, G_, S)), op=OP.mult)
                    # Q *= (1 + v)  as  Q += Q*v  (no scalar ops on Pool)
                    k2 = tl("g1")
                    pe_.tensor_tensor(out=k2[:], in0=Q, in1=k1[:],
                                      op=OP.mult)
                    pe_.tensor_tensor(out=Q, in0=Q, in1=k2[:], op=OP.add)
                    # A' = (1+bh)*A  (in place)
                    nc.vector.scalar_tensor_tensor(A, bh, 1.0, A,
                                                   op0=OP.add, op1=OP.mult)
                    # ut = bh / A'  (stored bf16 into the u history)
                    rA = tl("g2")       # g2 dead after r2m
                    nc.vector.reciprocal_approx_fast(rA[:], A)
                    ut = u4[:, :, :, t]
                    nc.vector.tensor_tensor(out=ut, in0=bh, in1=rA[:],
                                            op=OP.mult)
                    # qw' = qw + ut*(ut*Gtt - 2*d1)
                    # (gpsimd once steps are long enough to hide it)
                    qe = nc.gpsimd
                    gttv = gb4[:, c0:c0 + G_, t, t][:, :, None]
                    h1 = tl("h1")
                    qe.tensor_tensor(
                        out=h1[:], in0=ut,
                        in1=gttv.broadcast_to((128, G_, S)), op=OP.mult)
                    h2 = tl("h2")
                    qe.tensor_tensor(out=h2[:], in0=h1[:], in1=dd[:],
                                     op=OP.subtract)
                    h3 = tl("h1")       # h1 dead after h2
                    qe.tensor_tensor(out=h3[:], in0=ut, in1=h2[:],
                                     op=OP.mult)
                    qe.tensor_tensor(out=qw, in0=qw, in1=h3[:],
                                     op=OP.add)

                for t in range(T):
                    for g in range(len(GROUPS)):
                        flow_head(g, t)
                    for g in range(len(GROUPS)):
                        flow_mid(g, t)

            # =========================  epilogue  =========================
            # Pin all epilogue ACT work behind a single natural_log_exp table
            # load (Sqrt/Ln/Exp live in different sets).
            nle_id = list(get_activation_tables(nc.m.arch)).index(
                "natural_log_exp_and_others")
            tbl_load = mybir.InstLoadActFuncSet(
                name=f"I-{nc.next_id()}", act_func_set_id=nle_id, ins=[], outs=[])
            tl_bi = nc.scalar.add_instruction(tbl_load)
            add_dep_helper(tl_bi.ins, last_flow_act.ins, True,
                           "table load after flow phase")

            def act_pinned(out, in_, func, **kw):
                bi = nc.scalar.activation(out, in_, func, **kw)
                add_dep_helper(bi.ins, tl_bi.ins, True, "epilogue act after load")
                return bi

            with tc.tile_pool(name="epi", bufs=1) as ep:
                lpw = ep.tile([128, CP * S], F)
                lpw3 = lpw.rearrange("p (c s) -> p c s", s=S)
                # lpw = -0.5*A^2*qw + 63*ln(A) + ln(Q) + cadd
                # za/zq/zqc only need flow state -> gpsimd, ahead of the Lns
                za = ep.tile([128, CP, S], F)
                nc.vector.tensor_tensor(out=za[:, :, :], in0=A_t[:, :, :],
                                        in1=A_t[:, :, :], op=OP.mult)
                zq = ep.tile([128, CP, S], F)
                nc.vector.tensor_tensor(out=zq[:, :, :], in0=za[:, :, :],
                                        in1=qw_t[:, :, :], op=OP.mult)
                cv = cadd_sb[:, :, None]
                zqc = ep.tile([128, CP, S], F)
                nc.vector.scalar_tensor_tensor(
                    zqc[:, :, :], zq[:, :, :], -0.5,
                    cv.broadcast_to((128, CP, S)),
                    op0=OP.mult, op1=OP.add)
                l1 = ep.tile([128, CP * S], F)
                act_pinned(l1[:], A_t[:, :, :].rearrange("p c s -> p (c s)"),
                           AF.Ln)
                l2 = ep.tile([128, CP * S], F)
                act_pinned(l2[:], Q_t[:, :, :].rearrange("p c s -> p (c s)"),
                           AF.Ln)
                w1 = ep.tile([128, CP * S], F)
                nc.vector.scalar_tensor_tensor(w1[:], l1[:], float(D - 1), l2[:],
                                               op0=OP.mult, op1=OP.add)
                nc.vector.tensor_tensor(
                    out=lpw3, in0=w1.rearrange("p (c s) -> p c s", s=S),
                    in1=zqc[:, :, :], op=OP.add)

                lpw_perm = lpw.rearrange("p (c s) -> p s c", s=S)
                mscl = ep.tile([128, 3, S], F)
                mx = mscl[:, 0, :]
                se = mscl[:, 1, :]
                clsl = mscl[:, 2, :]
                nc.vector.tensor_reduce(mx, lpw_perm, axis=AX.X, op=OP.max)
                exs = ep.tile([128, CP * S], F)
                exs3 = exs.rearrange("p (c s) -> p c s", s=S)
                mx_b = mx[:, None, :].broadcast_to((128, CP, S))
                nc.vector.tensor_tensor(out=exs3, in0=lpw3[:, :, :], in1=mx_b,
                                        op=OP.subtract)
                act_pinned(exs[:], exs[:], AF.Exp)
                nc.vector.tensor_reduce(
                    se, exs.rearrange("p (c s) -> p s c", s=S),
                    axis=AX.X, op=OP.add)
                msk_sb = ep.tile([128, CP * S], F)
                nc.sync.dma_start(msk_sb[:], masksb[:])
                gsum = ep.tile([128, CP * S], F)   # own buffer: the mask
                # path runs in parallel with the sum-exp path
                nc.vector.tensor_tensor(out=gsum[:], in0=msk_sb[:], in1=lpw[:],
                                        op=OP.mult)
                nc.vector.tensor_reduce(
                    clsl, gsum.rearrange("p (c s) -> p s c", s=S),
                    axis=AX.X, op=OP.add)

                # ---- AllToAll: ccin[j] = (mx, se, cls) for sample-slice j ----
                ccin = dp.tile([NCORES, 3, 128 * SL], F)
                ccout = dp.tile([NCORES, 3, 128 * SL], F)
                ccin_v = ccin.rearrange("r t (p s) -> t p r s", p=128)
                for ti in range(3):
                    nc.sync.dma_start(
                        ccin_v[ti],
                        mscl[:, ti, :].rearrange("p (r s) -> p r s", s=SL))
                nc.gpsimd.collective_compute(
                    "AllToAll", OP.bypass,
                    replica_groups=[list(range(NCORES))],
                    ins=[ccin.opt()], outs=[ccout.opt()],
                )
                # ---- logits path fills the AllToAll wait ----
                corr_sb = ep.tile([128, SL], F)
                nc.sync.dma_start(corr_sb[:], corr_in[:])
                xsl_sb = ep.tile([D + 1, 128 * SL], F)
                nc.sync.dma_start(xsl_sb[:], xslice[:])
                Wb_sb = ep.tile([D + 1, C], F)
                nc.sync.dma_start(Wb_sb[:], Wb[:])
                lg = ep.tile([128, SL * C], F)
                for j in range(SL):
                    pl = plp.tile([128, C], F)
                    nc.tensor.matmul(pl[:],
                                     lhsT=xsl_sb[:, 128 * j:128 * (j + 1)],
                                     rhs=Wb_sb[:], start=True, stop=True)
                    nc.scalar.copy(lg[:, C * j:C * (j + 1)], pl[:])
                lg3 = lg.rearrange("p (s c) -> p s c", c=C)
                ml = ep.tile([128, SL], F)
                nc.vector.tensor_reduce(ml[:], lg3, axis=AX.X, op=OP.max)
                ml_b = ml[:, :, None].broadcast_to((128, SL, C))
                nc.vector.tensor_tensor(out=lg3, in0=lg3, in1=ml_b,
                                        op=OP.subtract)
                act_pinned(lg[:], lg[:], AF.Exp)
                ssum = ep.tile([128, SL], F)
                nc.vector.tensor_reduce(ssum[:], lg3, axis=AX.X, op=OP.add)
                rsum = ep.tile([128, SL], F)
                rscr = ep.tile([128, SL], F)
                nc.vector.reciprocal_approx_accurate(rsum[:], ssum[:], rscr[:])

                ccout_v = ccout.rearrange("r t (p s) -> t p r s", p=128)
                cco = ep.tile([128, 3, NCORES, SL], F)
                for ti in range(3):
                    nc.sync.dma_start(cco[:, ti], ccout_v[ti])
                mxg = cco[:, 0]
                seg = cco[:, 1]
                clg = cco[:, 2]

                # ---- global combine for our slice ----
                M = ep.tile([128, SL], F)
                nc.vector.tensor_reduce(M[:], mxg.rearrange("p r s -> p s r"),
                                        axis=AX.X, op=OP.max)
                esh = ep.tile([128, NCORES * SL], F)
                esh3 = esh.rearrange("p (r s) -> p r s", s=SL)
                M_b = M[:, None, :].broadcast_to((128, NCORES, SL))
                nc.vector.tensor_tensor(out=esh3, in0=mxg, in1=M_b,
                                        op=OP.subtract)
                act_pinned(esh[:], esh[:], AF.Exp)
                wsum = ep.tile([128, NCORES * SL], F)
                nc.vector.tensor_tensor(out=wsum[:], in0=esh[:], in1=seg.rearrange("p r s -> p (r s)"),
                                        op=OP.mult)
                Sg = ep.tile([128, SL], F)
                nc.vector.tensor_reduce(
                    Sg[:], wsum.rearrange("p (r s) -> p s r", s=SL),
                    axis=AX.X, op=OP.add)
                lse = ep.tile([128, SL], F)
                act_pinned(lse[:], Sg[:], AF.Ln)
                nc.vector.tensor_tensor(out=lse[:], in0=lse[:], in1=M[:],
                                        op=OP.add)
                clsf = ep.tile([128, SL], F)
                nc.vector.tensor_reduce(clsf[:], clg.rearrange("p r s -> p s r"),
                                        axis=AX.X, op=OP.add)
                nc.vector.tensor_tensor(out=clsf[:], in0=clsf[:], in1=corr_sb[:],
                                        op=OP.subtract)
                lev = ep.tile([128, SL], F)
                nc.vector.tensor_scalar(out=lev[:], in0=lse[:],
                                        scalar1=EV_BUDGET,
                                        scalar2=LOG_EV_CLAMP, op0=OP.add,
                                        op1=OP.min)
                ev = ep.tile([128, SL], F)
                act_pinned(ev[:], lev[:], AF.Exp)

                # ---- combine evidence with precomputed softmax ----
                evn = ep.tile([128, SL], F)
                nc.vector.tensor_tensor(out=evn[:], in0=ev[:], in1=rsum[:],
                                        op=OP.mult)
                evn_b = evn[:, :, None].broadcast_to((128, SL, C))
                t1 = lg  # in-place: exp(logits) no longer needed afterwards
                t13 = lg3
                nc.vector.tensor_tensor(out=t13, in0=lg3, in1=evn_b, op=OP.mult)
                la = gsum[:, :SL * C]  # gsum dead after the cls reduce
                act_pinned(la[:], t1[:], AF.Ln, bias=1.0)
                # accurate log1p for small x: x*(1 + x*(-1/2 + x/3)) when x<0.01
                h1e = ep.tile([128, SL * C], F)
                nc.vector.tensor_scalar(out=h1e[:], in0=t1[:], scalar1=1.0 / 3.0,
                                        scalar2=-0.5, op0=OP.mult, op1=OP.add)
                nc.vector.tensor_tensor(out=h1e[:], in0=h1e[:], in1=t1[:],
                                        op=OP.mult)
                nc.vector.tensor_scalar_add(h1e[:], h1e[:], 1.0)
                nc.vector.tensor_tensor(out=h1e[:], in0=h1e[:], in1=t1[:],
                                        op=OP.mult)
                h2e = h1e
                lmask = ep.tile([128, SL * C], mybir.dt.uint8)
                nc.vector.tensor_scalar(out=lmask[:], in0=t1[:], scalar1=0.01,
                                        scalar2=None, op0=OP.is_lt)
                nc.vector.select(la[:], lmask[:], h2e[:], la[:])

                ob = lpw[:, :SL * (C + 1)]  # lpw dead after gsum
                ob3 = ob.rearrange("p (s c) -> p s c", c=C + 1)
                la3 = la.rearrange("p (s c) -> p s c", c=C)
                out_v = out_d.rearrange("(s p) c -> p s c", p=128)
                H = SL // 2
                nc.vector.tensor_copy(ob3[:, :H, 0:C], la3[:, :H, :])
                nc.vector.tensor_copy(ob3[:, :H, C:C + 1], clsf[:, :H, None])
                nc.sync.dma_start(out_v[:, :H], ob3[:, :H, :])
                nc.vector.tensor_copy(ob3[:, H:, 0:C], la3[:, H:, :])
                nc.vector.tensor_copy(ob3[:, H:, C:C + 1], clsf[:, H:, None])
                nc.sync.dma_start(out_v[:, H:], ob3[:, H:, :])

    nc.finalize()
    return nc


def _softplus(v):
    return np.log1p(np.exp(-np.abs(v))) + np.maximum(v, 0)


def host_prep(x, labels, labels_frequency, z0, alpha_prime, beta_prime, W, b):
    import ml_dtypes
    x = np.asarray(x, np.float32)
    labels = np.asarray(labels).astype(np.int64)
    freq = np.asarray(labels_frequency, np.float32)
    z0 = np.asarray(z0, np.float32)
    alpha = _softplus(np.asarray(alpha_prime, np.float32)).astype(np.float32)
    beta = (-alpha + _softplus(np.asarray(beta_prime, np.float32))).astype(np.float32)
    W = np.asarray(W, np.float32)
    b = np.asarray(b, np.float32)

    xaugT = np.concatenate([x.T, np.ones((1, N), np.float32)], axis=0)  # [65, N]
    xbf = np.ascontiguousarray(x.T).astype(ml_dtypes.bfloat16)          # [D, N]
    Wb = np.concatenate([W, b[None, :]], axis=0).astype(np.float32)    # [65, C]
    xsq = np.sum(x * x, axis=1).astype(np.float32).reshape(S, 128).T   # [128, S]
    logfreq = np.log(freq).astype(np.float32)
    lab_ps = labels.reshape(S, 128).T                                  # [128, S]

    ones128 = np.ones((128, 1), np.float32)
    in_maps = []
    for k, (cls, real) in enumerate(_class_split()):
        z0c = z0[cls]                                   # [CP, T, D]
        alc = alpha[cls]                                # [CP, T]
        bec = beta[cls]
        G = np.einsum('cij,ckj->cik', z0c, z0c).astype(np.float32)   # [CP,T,T]
        n0 = np.sum(z0c * z0c, axis=2).astype(np.float32)            # [CP, T]
        Gb = np.broadcast_to(
            G.astype(ml_dtypes.bfloat16).reshape(CP, 1, T * T),
            (CP, 128, T * T)).copy()
        ab = (alc * bec).astype(np.float32)
        tabs = np.concatenate([alc.reshape(-1), bec.reshape(-1),
                               n0.reshape(-1), ab.reshape(-1)])
        tabs_rk = np.broadcast_to(tabs.reshape(1, 4 * CP * T),
                                  (128, 4 * CP * T)).copy()
        cadd = np.array([(logfreq[c] + NEG_HALF_DLOG2PI) if r else PAD_NEGINF
                         for c, r in zip(cls, real)], np.float32)
        cadd_rk = (ones128 * cadd[None, :]).astype(np.float32)
        msk = np.zeros((128, CP, S), np.float32)
        for i, (c, r) in enumerate(zip(cls, real)):
            if r:
                msk[:, i, :] = (lab_ps == c)
        sl = slice(1024 * k, 1024 * (k + 1))
        corr_k = logfreq[labels[sl]].reshape(SL, 128).T.astype(np.float32)
        in_maps.append(dict(
            xbf=xbf, xslice=np.ascontiguousarray(xaugT[:, sl]), Wb=Wb,
            xsq=xsq,
            z0T=np.ascontiguousarray(z0c.transpose(2, 0, 1)).reshape(
                D, CP * T).astype(ml_dtypes.bfloat16),
            Gb=Gb, tabs_r=tabs_rk,
            cadd_r=cadd_rk, corr=corr_k,
            masksb=msk.reshape(128, CP * S),
        ))
    return in_maps


def kernel(**inputs) -> np.ndarray:
    if "nc" not in _CACHE:
        _CACHE["nc"] = build_program()
    nc = _CACHE["nc"]
    in_maps = host_prep(**inputs)
    if os.environ.get("KERNEL_SIM"):
        from concourse.bass_interp import MultiCoreSim
        sim = MultiCoreSim(nc, NCORES)
        for k in range(NCORES):
            for name, arr in in_maps[k].items():
                sim.cores[k].tensor(name)[:] = arr
        sim.simulate()
        outs = [np.array(sim.cores[k].tensor("out")) for k in range(NCORES)]
    else:
        res = run_bass_kernel_spmd(nc, in_maps, list(range(NCORES)))
        outs = [res.results[k]["out"] for k in range(NCORES)]
    return np.concatenate(outs, axis=0)
